# revision 2
# baseline (speedup 1.0000x reference)
"""Causal attention kernel for TRN2, 8 NeuronCores (SPMD) — v2.

Problem:  x[4096,2048] f32; q = x@Wq.T, k = x@Wk.T (d_head=128),
          scores = q@k.T causal-masked, attn = softmax(scores),
          out = (attn @ x) @ W2.T.

Sharding: sequence-parallel over queries with stride-8 interleave:
  core c owns queries {8m+c : m=0..511}.  For key tile kt (128 keys),
  every core has exactly 512-16*kt valid queries (a contiguous tail
  slice of its query columns), so the SPMD program is identical on all
  cores (no collectives) and causal work is perfectly balanced.

v2 changes vs the 215us baseline (traced on HW):
  * x is shipped ONCE (fp16, natural layout, 16.8MB) instead of twice
    (transposed fp16 for the k-projection + natural bf16 for the V
    matmul = 33.6MB).  The transposed copy is derived on-chip with
    xbar DMA-transposes (SBUF->SBUF on the scalar HWDGE queue), so
    HBM traffic drops 45MB -> 28MB/core and the entire post-softmax
    phase runs from resident x instead of being paced by the xv
    stream (which cost ~15us of PE idle + 2 HAM re-throttles).
  * kT projection consumes 512-key staging groups (matmul N=512
    instead of N=128): 128 LDWEIGHTS+MM pairs instead of 512.
  * V chunks oc0-4 are fused into the score loop over the FULL causal
    range (PSUM: kT 1 + scores 1 + denom 1 + V 5 = 8 banks); the
    remaining chunks run as three 4(3)-wide waves on rotating banks.
  * Byte schedule starts with wk + x group 0 so the PE begins kT
    projection ~5us earlier; softmax reciprocal is computed on the
    [128,512] broadcast grid (the [1,512] one-lane reciprocal was
    3.3us of critical path).

Precision: fp16 x/weights everywhere; scores matmul fp16 (fp32 PSUM),
  unnormalized softmax in bf16 (fp32 exponent range for exp(s) up to
  ~1e30), V matmul = fp16 stationary x bf16 moving, normalization at
  eviction.
"""

from contextlib import ExitStack

import numpy as np

import concourse.bass as bass
import concourse.bacc as bacc
import concourse.mybir as mybir
import concourse.tile as tile
from concourse.bass_utils import run_bass_kernel_spmd

N_CTX = 4096
D_MODEL = 2048
D_HEAD = 128
NCORES = 8
QPC = N_CTX // NCORES          # 512 queries per core
NKT = N_CTX // 128             # 32 key tiles
NDM = D_MODEL // 128           # 16 d_model chunks
SUBS = 4                       # key tiles per staging group
KG = SUBS * 128                # 512 keys per staging group
NKG = N_CTX // KG              # 8 staging groups
NFUSE = 5                      # V output chunks fused into the score loop
MASK_NEG = -1.0e30

F16 = mybir.dt.float16
BF16 = mybir.dt.bfloat16
F32 = mybir.dt.float32


def _widths():
    # valid query-column width per key tile (tail slice [512-w : 512] of qT)
    return [QPC - 16 * kt for kt in range(NKT)]


def build_program():
    nc = bacc.Bacc(trn_type="TRN2", target_bir_lowering=False, debug=False)

    # ---- DRAM parameters (identical shapes on all cores; data differs) ----
    # xnr[kg, p, SUBS*d + ...]: xnr[kg, p, sub*D_MODEL + d] = x[512*kg + 128*sub + p, d]
    xnr = nc.declare_dram_parameter("xnr", [NKG, 128, SUBS * D_MODEL], F16, isOutput=False)
    # xqr[r, 512*ic + m] = x[8m+c, 128*ic + r]   (own-query columns, packed)
    xqr = nc.declare_dram_parameter("xqr", [128, NDM * QPC], F16, isOutput=False)
    # wqr[r, 128*ic + h] = Wq[h, 128*ic + r]; same for wkr
    wqr = nc.declare_dram_parameter("wqr", [128, D_MODEL], F16, isOutput=False)
    wkr = nc.declare_dram_parameter("wkr", [128, D_MODEL], F16, isOutput=False)
    # w2r[oc][r, 128*ic + o] = W2[128*oc + o, 128*ic + r]
    w2r = nc.declare_dram_parameter("w2r", [NDM, 128, D_MODEL], F16, isOutput=False)
    maskb = nc.declare_dram_parameter("maskb", [128, 16], F32, isOutput=False)
    outT = nc.declare_dram_parameter("outT", [D_MODEL, QPC], F16, isOutput=True)

    W = _widths()

    with tile.TileContext(nc) as tc:
        with (
            tc.tile_pool(name="static", bufs=1) as st,
            tc.tile_pool(name="xnpool", bufs=NKG) as xnp,
            tc.tile_pool(name="atpool", bufs=1) as atp,
        ):
            qT_sb = st.tile([128, QPC], F16, tag="qT")
            ones_sb = st.tile([128, 1], BF16, tag="ones")
            mask_sb = st.tile([128, 16], F32, tag="mask")
            sums_sb = st.tile([128, QPC], F32, tag="sums")
            recip_sb = st.tile([128, QPC], F32, tag="recip")
            nc.vector.memset(ones_sb[:], 1.0)

            # ---- byte schedule (sync queue order == emission order):
            # mask, wk, x group 0, wq, xqr, x groups 1-7, w2r (in W2 loop).
            nc.sync.dma_start(out=mask_sb[:], in_=maskb[:])

            es2 = ExitStack()  # fused-phase SBUF: wk + staging + ktile
            pk = es2.enter_context(tc.tile_pool(name="pk", bufs=1))
            wk_sb = pk.tile([128, D_MODEL], F16, tag="wk")
            nc.sync.dma_start(out=wk_sb[:], in_=wkr[:])

            xn_g = []
            for kg in range(NKG):
                t = xnp.tile([128, SUBS * D_MODEL], F16, tag="xn", name=f"xng{kg}")
                xn_g.append(t)
            nc.sync.dma_start(out=xn_g[0][:], in_=xnr[0])

            def xblk(kt, oc):
                # natural-layout x block [128 keys, 128 d] for key tile kt,
                # d chunk oc (V-matmul stationary)
                base = (kt % SUBS) * D_MODEL + 128 * oc
                return xn_g[kt // SUBS][:, base : base + 128]

            es1 = ExitStack()  # q-projection SBUF transients
            p1 = es1.enter_context(tc.tile_pool(name="p1", bufs=1))
            wq_sb = p1.tile([128, D_MODEL], F16, tag="wq")
            nc.sync.dma_start(out=wq_sb[:], in_=wqr[:])
            xq_sb = p1.tile([128, NDM * QPC], F16, tag="xq")
            for qq in range(4):
                nc.sync.dma_start(
                    out=xq_sb[:, 4 * QPC * qq : 4 * QPC * (qq + 1)],
                    in_=xqr[:, 4 * QPC * qq : 4 * QPC * (qq + 1)],
                )
            for kg in range(1, NKG):
                nc.sync.dma_start(out=xn_g[kg][:], in_=xnr[kg])

            # ---- qT projection ----
            with tc.tile_pool(name="psq", bufs=1, space="PSUM") as psqp:
                psq = psqp.tile([128, QPC], F32, tag="psq")
                for ic in range(NDM):
                    nc.tensor.matmul(
                        psq[:],
                        wq_sb[:, 128 * ic : 128 * (ic + 1)],
                        xq_sb[:, QPC * ic : QPC * (ic + 1)],
                        start=(ic == 0), stop=(ic == NDM - 1),
                    )
                nc.vector.tensor_copy(qT_sb[:], psq[:])
            es1.close()

            # ---- fused pipeline: transpose / kT proj / scores / exp /
            #      denom / V[0:NFUSE] over the full causal range ----
            es3 = ExitStack()  # PSUM: fused V accumulators (freed post-evict)
            psv = es3.enter_context(
                tc.tile_pool(name="psv", bufs=NFUSE, space="PSUM", side="right")
            )
            pso1 = [
                psv.tile([128, QPC], F32, tag="pso", name=f"pso{j}")
                for j in range(NFUSE)
            ]
            xts = es2.enter_context(tc.tile_pool(name="xts", bufs=2))
            ktp = es2.enter_context(tc.tile_pool(name="ktp", bufs=2))
            at_t = []
            with (
                tc.tile_pool(name="psk", bufs=1, space="PSUM") as pskp,
                tc.tile_pool(name="pss", bufs=1, space="PSUM") as pssp,
                tc.tile_pool(name="psd", bufs=1, space="PSUM") as psdp,
            ):
                psd = psdp.tile([1, QPC], F32, tag="psd")
                for kg in range(NKG):
                    # on-chip transpose: stg[c, sub, ic, p] = x[512kg+128sub+p, 128ic+c]
                    stg = xts.tile([128, SUBS, NDM, 128], F16, tag="stg", name=f"stg{kg}")
                    for sub in range(SUBS):
                        nc.scalar.dma_start_transpose(
                            stg[:, sub, :, :],
                            xn_g[kg][:, sub * D_MODEL : (sub + 1) * D_MODEL],
                        )
                    psk = pskp.tile([128, KG], F32, tag="psk", name=f"psk{kg}")
                    for ic in range(NDM):
                        nc.tensor.matmul(
                            psk[:],
                            wk_sb[:, 128 * ic : 128 * (ic + 1)],
                            stg[:, :, ic, :],
                            start=(ic == 0), stop=(ic == NDM - 1),
                        )
                    ktile = ktp.tile([128, KG], F16, tag="kt", name=f"kt{kg}")
                    nc.vector.tensor_copy(ktile[:], psk[:])

                    for sub in range(SUBS):
                        kt = kg * SUBS + sub
                        w = W[kt]
                        ps = pssp.tile([128, 512], F32, tag="pss", name=f"pss{kt}")
                        nc.tensor.matmul(
                            ps[:, :w],
                            ktile[:, 128 * sub : 128 * (sub + 1)],
                            qT_sb[:, QPC - w : QPC],
                            start=True, stop=True,
                        )
                        nc.vector.tensor_add(ps[:, :16], ps[:, :16], mask_sb[:])
                        at = atp.tile([128, w], BF16, tag=f"at{kt}")
                        nc.scalar.activation(
                            at[:], ps[:, :w], mybir.ActivationFunctionType.Exp
                        )
                        at_t.append(at)
                        nc.tensor.matmul(
                            psd[0:1, QPC - w : QPC],
                            ones_sb[:],
                            at[:],
                            start=(kt == 0), stop=(kt == NKT - 1),
                        )
                        for j in range(NFUSE):
                            nc.tensor.matmul(
                                pso1[j][:, QPC - w : QPC],
                                xblk(kt, j),
                                at[:],
                                start=(kt == 0), stop=(kt == NKT - 1),
                            )
                # softmax denominators out of PSUM before the pool closes
                nc.vector.tensor_copy(sums_sb[0:1, :], psd[0:1, :])
            es2.close()  # free wk + staging + ktile SBUF

            nc.gpsimd.partition_broadcast(sums_sb[:], sums_sb[0:1, :])
            nc.vector.reciprocal(recip_sb[:], sums_sb[:])

            with tc.tile_pool(name="p34", bufs=1) as p34:
                ao_t = {}

                def evict(oc, src):
                    t = p34.tile([128, QPC], F16, tag=f"ao{oc}", name=f"ao{oc}")
                    nc.vector.tensor_mul(t[:], src[:], recip_sb[:])
                    ao_t[oc] = t

                # ---- wave A: oc NFUSE..8 on the banks freed by psk/pss/psd
                esA = ExitStack()
                gA = esA.enter_context(
                    tc.tile_pool(name="gA", bufs=8 - NFUSE, space="PSUM")
                )
                A = {
                    oc: gA.tile([128, QPC], F32, tag="gA", name=f"gA{oc}")
                    for oc in range(NFUSE, 8)
                }
                for kt in range(NKT):
                    w = W[kt]
                    for oc in range(NFUSE, 8):
                        nc.tensor.matmul(
                            A[oc][:, QPC - w : QPC],
                            xblk(kt, oc),
                            at_t[kt][:],
                            start=(kt == 0), stop=(kt == NKT - 1),
                        )
                for j in range(NFUSE):
                    evict(j, pso1[j])
                es3.close()  # free the fused V banks

                # ---- wave B: oc 8-11
                esB = ExitStack()
                gB = esB.enter_context(
                    tc.tile_pool(name="gB", bufs=4, space="PSUM", side="right")
                )
                B = {
                    oc: gB.tile([128, QPC], F32, tag="gB", name=f"gB{oc}")
                    for oc in range(8, 12)
                }
                for kt in range(NKT):
                    w = W[kt]
                    for oc in range(8, 12):
                        nc.tensor.matmul(
                            B[oc][:, QPC - w : QPC],
                            xblk(kt, oc),
                            at_t[kt][:],
                            start=(kt == 0), stop=(kt == NKT - 1),
                        )
                for oc in range(NFUSE, 8):
                    evict(oc, A[oc])
                esA.close()

                # ---- wave C: oc 12-15
                esC = ExitStack()
                gC = esC.enter_context(
                    tc.tile_pool(name="gC", bufs=4, space="PSUM")
                )
                C = {
                    oc: gC.tile([128, QPC], F32, tag="gC", name=f"gC{oc}")
                    for oc in range(12, 16)
                }
                for kt in range(NKT):
                    w = W[kt]
                    for oc in range(12, 16):
                        nc.tensor.matmul(
                            C[oc][:, QPC - w : QPC],
                            xblk(kt, oc),
                            at_t[kt][:],
                            start=(kt == 0), stop=(kt == NKT - 1),
                        )
                for oc in range(8, 12):
                    evict(oc, B[oc])
                esB.close()
                for oc in range(12, 16):
                    evict(oc, C[oc])
                esC.close()

                # ---- W2: outT = W2T.T @ attn_outT ----
                with (
                    tc.tile_pool(name="w2s", bufs=4) as w2s,
                    tc.tile_pool(name="outs", bufs=4) as outs,
                    tc.tile_pool(name="ps4", bufs=6, space="PSUM", side="right") as ps4p,
                ):
                    for oc in range(NDM):
                        tw = w2s.tile([128, D_MODEL], F16, tag="w2")
                        nc.sync.dma_start(out=tw[:], in_=w2r[oc])
                        ps = ps4p.tile([128, QPC], F32, tag="ps4")
                        for ic in range(NDM):
                            nc.tensor.matmul(
                                ps[:],
                                tw[:, 128 * ic : 128 * (ic + 1)],
                                ao_t[ic][:],
                                start=(ic == 0), stop=(ic == NDM - 1),
                            )
                        t = outs.tile([128, QPC], F16, tag="out")
                        nc.vector.tensor_copy(t[:], ps[:])
                        nc.scalar.dma_start(
                            out=outT[128 * oc : 128 * (oc + 1), :], in_=t[:]
                        )

    nc.compile()
    return nc


def prepare_inputs(x, Wk, Wq, W2):
    """Host-side sharding/layout prep. Returns in_maps for the 8 cores."""
    x = np.asarray(x, dtype=np.float32)
    Wk = np.asarray(Wk, dtype=np.float32)
    Wq = np.asarray(Wq, dtype=np.float32)
    W2 = np.asarray(W2, dtype=np.float32)

    x16 = x.astype(np.float16)
    # xnr[kg, p, sub*D_MODEL + d] = x[512*kg + 128*sub + p, d]
    xnr = np.ascontiguousarray(
        x16.reshape(NKG, SUBS, 128, D_MODEL).transpose(0, 2, 1, 3).reshape(
            NKG, 128, SUBS * D_MODEL
        )
    )

    def pack_chunks(aT, width):
        # aT [D_MODEL, width] -> [128, NDM*width]: out[r, width*ic + c] = aT[128ic+r, c]
        return np.ascontiguousarray(
            aT.reshape(NDM, 128, width).transpose(1, 0, 2).reshape(128, NDM * width)
        )

    wqr = pack_chunks(np.ascontiguousarray(Wq.T).astype(np.float16), D_HEAD)
    wkr = pack_chunks(np.ascontiguousarray(Wk.T).astype(np.float16), D_HEAD)
    # w2r[oc, r, 128*ic + o] = W2T[128ic+r, 128oc+o]
    w2T = np.ascontiguousarray(W2.T).astype(np.float16)
    w2r = np.ascontiguousarray(
        w2T.reshape(NDM, 128, NDM, 128).transpose(2, 1, 0, 3).reshape(NDM, 128, D_MODEL)
    )

    in_maps = []
    for c in range(NCORES):
        xqT = np.ascontiguousarray(x[c::NCORES].T).astype(np.float16)  # [D, QPC]
        xqr_c = pack_chunks(xqT, QPC)
        mask = np.zeros((128, 16), dtype=np.float32)
        j = np.arange(128)[:, None]
        t = np.arange(16)[None, :]
        mask[j > 8 * t + c] = MASK_NEG
        in_maps.append(
            {
                "xnr": xnr,
                "xqr": xqr_c,
                "wqr": wqr,
                "wkr": wkr,
                "w2r": w2r,
                "maskb": mask,
            }
        )
    return in_maps


def assemble_output(results):
    res = np.stack([np.asarray(results[c]["outT"]).astype(np.float32) for c in range(NCORES)])
    # [c, d, m] -> out[8m+c, d]
    return np.ascontiguousarray(res.transpose(2, 0, 1).reshape(N_CTX, D_MODEL))


_CACHED = {}


def kernel(x, Wk, Wq, W2, _trace=False):
    if "nc" not in _CACHED:
        _CACHED["nc"] = build_program()
    nc = _CACHED["nc"]
    in_maps = prepare_inputs(x, Wk, Wq, W2)
    res = run_bass_kernel_spmd(nc, in_maps, core_ids=list(range(NCORES)), trace=_trace)
    out = assemble_output(res.results)
    if _trace:
        return out, res
    return out


# revision 7
# speedup vs baseline: 1.3105x; 1.3105x over previous
"""Causal attention kernel for TRN2, 8 NeuronCores (SPMD) — v3.

Problem:  x[4096,2048] f32; q = x@Wq.T, k = x@Wk.T (d_head=128),
          scores = q@k.T causal-masked, attn = softmax(scores),
          out = (attn @ x) @ W2.T.

Sharding: sequence-parallel over queries with stride-8 interleave:
  core c owns queries {8m+c : m=0..511}.  For key tile kt (128 keys),
  every core has exactly 512-16*kt valid queries -- a contiguous tail
  slice of its query columns -- so the SPMD program is identical on all
  cores (no dynamic control flow, no collectives) and causal work is
  perfectly balanced.

Precision: fp16 inputs for the q/k projections and the score matmul
  (fp32 PSUM accumulation), unnormalized softmax (exp without
  max-subtraction), attention weights in bf16 (fp32 exponent range,
  needed for exp(s) up to ~1e28), V and W2 matmuls in bf16/fp16,
  normalization by the softmax row-sum applied at eviction.

v3 scheduling (vs the 215us v1 baseline, trace-driven):
  * Dual-queue DMA: the xtp / xv / w2r streams alternate between the
    sync and scalar HWDGE queues (even tiles on sync, odd on scalar).
    One queue measured ~268 GB/s effective (issue-gap limited); two
    sequencers raise aggregate toward the ~358 GB/s HBM limit, which
    shortens the DMA-paced score window and starts the V phase
    earlier.  Per-queue emission order preserves the byte schedule
    (small loads -> xtp -> xv -> w2r) and xv stays ascending.
  * q-path small loads (wq, xqr) go on the scalar queue concurrently
    with wk+xtp0 on sync, so the kT projection starts ~5us earlier
    and the PE warms up immediately.
  * kT projection in 512-key groups (matmul N=512 instead of N=128):
    128 LDWEIGHTS+MM pairs instead of 512, saving ~13us of PE time,
    which matters once the score window is no longer DMA-bound.
  * Softmax reciprocal on the [128,512] broadcast grid (the [1,512]
    single-lane reciprocal was 3.3us of critical path).
  * V-matmul phase structure is v1's (PSUM-bank-optimal): fused oc0-3
    kt<16, g2 oc4-7 kt<16, bc oc8-15 full range, g4 oc0-7 kt>=16 with
    normalized merges; W2 interleaves via ic_order 8-15 first.
"""

from contextlib import ExitStack

import numpy as np
import ml_dtypes

import concourse.bass as bass
import concourse.bacc as bacc
import concourse.mybir as mybir
import concourse.tile as tile
from concourse.bass_utils import run_bass_kernel_spmd

N_CTX = 4096
D_MODEL = 2048
D_HEAD = 128
NCORES = 8
QPC = N_CTX // NCORES          # 512 queries per core
NKT = N_CTX // 128             # 32 key tiles
NDM = D_MODEL // 128           # 16 d_model chunks
SUBS = 4                       # key tiles per projection group
KG = SUBS * 128                # 512 keys per projection group
NKG = N_CTX // KG              # 8 projection groups
MASK_NEG = -1.0e30

F16 = mybir.dt.float16
BF16 = mybir.dt.bfloat16
F32 = mybir.dt.float32


def _widths():
    # valid query-column width per key tile (tail slice [512-w : 512] of qT)
    return [QPC - 16 * kt for kt in range(NKT)]


def build_program():
    nc = bacc.Bacc(trn_type="TRN2", target_bir_lowering=False, debug=False)

    # ---- DRAM parameters (identical shapes on all cores; data differs) ----
    # xqr[r, 512*ic + m] = x[8m+c, 128*ic + r]   (own-query columns, packed)
    xqr = nc.declare_dram_parameter("xqr", [128, NDM * QPC], F16, isOutput=False)
    # xtp[kg][r, KG*ic + n] = x[KG*kg + n, 128*ic + r]  (contiguous per-kg tiles)
    xtp = nc.declare_dram_parameter("xtp", [NKG, 128, NDM * KG], F16, isOutput=False)
    # xv = x (natural layout), bf16
    xv = nc.declare_dram_parameter("xv", [N_CTX, D_MODEL], BF16, isOutput=False)
    # wqr[r, 128*ic + h] = Wq[h, 128*ic + r]; same for wkr
    wqr = nc.declare_dram_parameter("wqr", [128, D_MODEL], F16, isOutput=False)
    wkr = nc.declare_dram_parameter("wkr", [128, D_MODEL], F16, isOutput=False)
    # w2r[oc][r, 128*ic + o] = W2[128*oc + o, 128*ic + r]
    w2r = nc.declare_dram_parameter("w2r", [NDM, 128, D_MODEL], F16, isOutput=False)
    maskb = nc.declare_dram_parameter("maskb", [128, 16], F32, isOutput=False)
    outT = nc.declare_dram_parameter("outT", [D_MODEL, QPC], F16, isOutput=True)

    W = _widths()

    def q_dma(i):
        # alternate big-stream tiles across the two HWDGE queues
        return nc.sync if i % 2 == 0 else nc.scalar

    with tile.TileContext(nc) as tc:
        with (
            tc.tile_pool(name="static", bufs=1) as st,
            tc.tile_pool(name="xvpool", bufs=NKT) as xvp,
            tc.tile_pool(name="atpool", bufs=1) as atp,
        ):
            qT_sb = st.tile([128, QPC], F16, tag="qT")
            ones_sb = st.tile([128, 1], BF16, tag="ones")
            mask_sb = st.tile([128, 16], F32, tag="mask")
            recip_sb = st.tile([128, QPC], F32, tag="recip")
            nc.vector.memset(ones_sb[:], 1.0)

            # ---- critical small loads: k-path on sync, q-path on scalar
            # (concurrent queues), then the big streams alternate.
            nc.sync.dma_start(out=mask_sb[:], in_=maskb[:])

            es1 = ExitStack()  # SBUF transients: p1 + xts (freed before p34)
            p1 = es1.enter_context(tc.tile_pool(name="p1", bufs=1))
            wk_sb = p1.tile([128, D_MODEL], F16, tag="wk")
            nc.sync.dma_start(out=wk_sb[:], in_=wkr[:])
            wq_sb = p1.tile([128, D_MODEL], F16, tag="wq")
            nc.scalar.dma_start(out=wq_sb[:], in_=wqr[:])
            xq_sb = p1.tile([128, NDM * QPC], F16, tag="xq")
            for qq in range(4):
                nc.scalar.dma_start(
                    out=xq_sb[:, 4 * QPC * qq : 4 * QPC * (qq + 1)],
                    in_=xqr[:, 4 * QPC * qq : 4 * QPC * (qq + 1)],
                )

            # ---- xT stream: one contiguous 2MB DMA per 512-key group,
            # alternating queues ----
            xts = es1.enter_context(tc.tile_pool(name="xts", bufs=2))
            ktp = es1.enter_context(tc.tile_pool(name="ktp", bufs=2))
            xts_t = []
            for kg in range(NKG):
                t = xts.tile([128, NDM * KG], F16, tag="xts", name=f"xts{kg}")
                q_dma(kg).dma_start(out=t[:], in_=xtp[kg])
                xts_t.append(t)

            # ---- xv stream: after xtp per queue (emission order), ascending
            xv_t = []
            for kt in range(NKT):
                t = xvp.tile([128, D_MODEL], BF16, tag="xv", name=f"xv{kt}")
                q_dma(kt).dma_start(out=t[:], in_=xv[128 * kt : 128 * (kt + 1), :])
                xv_t.append(t)

            # ---- qT projection ----
            with tc.tile_pool(name="psq", bufs=1, space="PSUM") as psqp:
                psq = psqp.tile([128, QPC], F32, tag="psq")
                for ic in range(NDM):
                    nc.tensor.matmul(
                        psq[:],
                        wq_sb[:, 128 * ic : 128 * (ic + 1)],
                        xq_sb[:, QPC * ic : QPC * (ic + 1)],
                        start=(ic == 0), stop=(ic == NDM - 1),
                    )
                nc.vector.tensor_copy(qT_sb[:], psq[:])

            # ---- fused pipeline: kT proj / scores / exp / denom / V[0:4] ----
            es2 = ExitStack()  # PSUM: psv1 (freed mid-way through V chunk waves)
            psv1 = es2.enter_context(
                tc.tile_pool(name="psv1", bufs=4, space="PSUM", side="right")
            )
            pso1 = [
                psv1.tile([128, QPC], F32, tag="pso1", name=f"pso1_{j}")
                for j in range(4)
            ]
            at_t = []
            with (
                tc.tile_pool(name="psk", bufs=2, space="PSUM") as pskp,
                tc.tile_pool(name="pss", bufs=1, space="PSUM") as pssp,
                tc.tile_pool(name="psd", bufs=1, space="PSUM") as psdp,
            ):
                psd = psdp.tile([1, QPC], F32, tag="psd")
                for kg in range(NKG):
                    psk = pskp.tile([128, KG], F32, tag="psk", name=f"psk{kg}")
                    for ic in range(NDM):
                        nc.tensor.matmul(
                            psk[:],
                            wk_sb[:, 128 * ic : 128 * (ic + 1)],
                            xts_t[kg][:, KG * ic : KG * (ic + 1)],
                            start=(ic == 0), stop=(ic == NDM - 1),
                        )
                    ktile = ktp.tile([128, KG], F16, tag="ktile", name=f"kt{kg}")
                    nc.vector.tensor_copy(ktile[:], psk[:])

                    for sub in range(SUBS):
                        kt = kg * SUBS + sub
                        w = W[kt]
                        ps = pssp.tile([128, 512], F32, tag="pss", name=f"pss{kt}")
                        nc.tensor.matmul(
                            ps[:, :w],
                            ktile[:, 128 * sub : 128 * (sub + 1)],
                            qT_sb[:, QPC - w : QPC],
                            start=True, stop=True,
                        )
                        nc.vector.tensor_add(ps[:, :16], ps[:, :16], mask_sb[:])
                        at = atp.tile([128, w], BF16, tag=f"at{kt}")
                        nc.scalar.activation(
                            at[:], ps[:, :w], mybir.ActivationFunctionType.Exp
                        )
                        at_t.append(at)
                        nc.tensor.matmul(
                            psd[0:1, QPC - w : QPC],
                            ones_sb[:],
                            at[:],
                            start=(kt == 0), stop=(kt == NKT - 1),
                        )
                        # V matmul for output chunks 0-3, fused
                        # (first key half only; the early eviction at xv[15]
                        # frees all 8 banks for the oc 8-15 full streams)
                        if kt < NKT // 2:
                            for j in range(4):
                                nc.tensor.matmul(
                                    pso1[j][:, QPC - w : QPC],
                                    xv_t[kt][:, 128 * j : 128 * (j + 1)],
                                    at[:],
                                    start=(kt == 0), stop=(kt == NKT // 2 - 1),
                                )

                # softmax denominators -> SBUF (reads psd before the pool
                # closes); broadcast + full-grid reciprocal follow
                nc.vector.tensor_copy(recip_sb[0:1, :], psd[0:1, :])

            es1.close()  # free p1 + xts SBUF for the aoT / W2 pools
            nc.gpsimd.partition_broadcast(recip_sb[:], recip_sb[0:1, :])
            nc.vector.reciprocal(recip_sb[:], recip_sb[:])

            with tc.tile_pool(name="p34", bufs=1) as p34:
                ao_t = {}

                # ---- oc 4-7, first key half (kt 0-15): runs concurrently
                # with the fused V[0:3] streams on the banks freed by the
                # fused pools; both finish when xv[15] lands ----
                HK = NKT // 2
                with tc.tile_pool(name="g2", bufs=4, space="PSUM") as g2p:
                    g2 = {
                        oc: g2p.tile([128, QPC], F32, tag="g2", name=f"g2_{oc}")
                        for oc in range(4, 8)
                    }
                    for kt in range(HK):
                        w = W[kt]
                        for oc in range(4, 8):
                            nc.tensor.matmul(
                                g2[oc][:, QPC - w : QPC],
                                xv_t[kt][:, 128 * oc : 128 * (oc + 1)],
                                at_t[kt][:],
                                start=(kt == 0), stop=(kt == HK - 1),
                            )
                    # early normalized evictions for oc 0-7 (partial over the
                    # first key half; exact for queries m<256 by causality)
                    for j in range(4):
                        t = p34.tile([128, QPC], F16, tag=f"ao{j}")
                        nc.vector.tensor_mul(t[:], pso1[j][:], recip_sb[:])
                        ao_t[j] = t
                    es2.close()  # release the fused V banks
                    for oc in range(4, 8):
                        t = p34.tile([128, QPC], F16, tag=f"ao{oc}")
                        nc.vector.tensor_mul(t[:], g2[oc][:], recip_sb[:])
                        ao_t[oc] = t

                # ---- oc 8-15, FULL key range: 8 concurrent streams on the
                # freed banks; kt 0-15 runs dense from resident xv while the
                # xv tail streams in ----
                with (
                    tc.tile_pool(name="bcL", bufs=4, space="PSUM") as bcL,
                    tc.tile_pool(name="bcR", bufs=4, space="PSUM", side="right") as bcR,
                ):
                    bc = {}
                    for i, oc in enumerate(range(8, NDM)):
                        pool = bcL if i < 4 else bcR
                        bc[oc] = pool.tile(
                            [128, QPC], F32, tag="bc", name=f"bc_{oc}"
                        )
                    for kt in range(NKT):
                        w = W[kt]
                        for oc in range(8, NDM):
                            nc.tensor.matmul(
                                bc[oc][:, QPC - w : QPC],
                                xv_t[kt][:, 128 * oc : 128 * (oc + 1)],
                                at_t[kt][:],
                                start=(kt == 0), stop=(kt == NKT - 1),
                            )
                    for oc in range(8, NDM):
                        t = p34.tile([128, QPC], F16, tag=f"ao{oc}")
                        nc.vector.tensor_mul(t[:], bc[oc][:], recip_sb[:])
                        ao_t[oc] = t

                # ---- oc 0-7, second key half (kt 16-31, queries [256:512]
                # only): dense from resident xv, merged into the early aos ----
                with (
                    tc.tile_pool(name="g4", bufs=4, space="PSUM") as g4p,
                    tc.tile_pool(name="tmr", bufs=4) as tmr,
                ):
                    for ocs in (range(0, 4), range(4, 8)):
                        g4 = {
                            oc: g4p.tile(
                                [128, QPC // 2], F32, tag="g4", name=f"g4_{oc}"
                            )
                            for oc in ocs
                        }
                        for kt in range(HK, NKT):
                            w = W[kt]
                            for oc in ocs:
                                nc.tensor.matmul(
                                    g4[oc][:, QPC // 2 - w : QPC // 2],
                                    xv_t[kt][:, 128 * oc : 128 * (oc + 1)],
                                    at_t[kt][:],
                                    start=(kt == HK), stop=(kt == NKT - 1),
                                )
                        for oc in ocs:
                            tm = tmr.tile([128, QPC // 2], F16, tag="tm")
                            nc.vector.tensor_mul(
                                tm[:], g4[oc][:], recip_sb[:, QPC // 2 :]
                            )
                            nc.vector.tensor_add(
                                ao_t[oc][:, QPC // 2 :],
                                ao_t[oc][:, QPC // 2 :],
                                tm[:],
                            )

                    # ---- W2: outT = W2T.T @ attn_outT.  ic order 8..15 first
                    # (those aos finish earliest), 0..7 after the merges ----
                    with (
                        tc.tile_pool(name="w2s", bufs=4) as w2s,
                        tc.tile_pool(name="outs", bufs=8) as outs,
                        tc.tile_pool(name="ps4", bufs=4, space="PSUM", side="right") as ps4,
                    ):
                        ic_order = list(range(8, NDM)) + list(range(0, 8))
                        for oc in range(NDM):
                            tw = w2s.tile([128, D_MODEL], F16, tag="w2")
                            q_dma(oc).dma_start(out=tw[:], in_=w2r[oc])
                            ps = ps4.tile([128, QPC], F32, tag="ps4")
                            for i, ic in enumerate(ic_order):
                                nc.tensor.matmul(
                                    ps[:],
                                    tw[:, 128 * ic : 128 * (ic + 1)],
                                    ao_t[ic][:],
                                    start=(i == 0), stop=(i == NDM - 1),
                                )
                            t = outs.tile([128, QPC], F16, tag="out")
                            nc.vector.tensor_copy(t[:], ps[:])
                            nc.scalar.dma_start(
                                out=outT[128 * oc : 128 * (oc + 1), :], in_=t[:]
                            )

    nc.compile()
    return nc


def prepare_inputs(x, Wk, Wq, W2):
    """Host-side sharding/layout prep. Returns in_maps for the 8 cores."""
    x = np.asarray(x, dtype=np.float32)
    Wk = np.asarray(Wk, dtype=np.float32)
    Wq = np.asarray(Wq, dtype=np.float32)
    W2 = np.asarray(W2, dtype=np.float32)

    xT16 = np.ascontiguousarray(x.T).astype(np.float16)          # [D, N]
    # xtp[kg, r, KG*ic + n] = xT[128*ic + r, KG*kg + n]
    xtp = np.ascontiguousarray(
        xT16.reshape(NDM, 128, NKG, KG).transpose(2, 1, 0, 3).reshape(NKG, 128, NDM * KG)
    )
    xv16 = x.astype(ml_dtypes.bfloat16)                          # [N, D]

    def pack_chunks(aT, width):
        # aT [D_MODEL, width] -> [128, NDM*width]: out[r, width*ic + c] = aT[128ic+r, c]
        return np.ascontiguousarray(
            aT.reshape(NDM, 128, width).transpose(1, 0, 2).reshape(128, NDM * width)
        )

    wqr = pack_chunks(np.ascontiguousarray(Wq.T).astype(np.float16), D_HEAD)
    wkr = pack_chunks(np.ascontiguousarray(Wk.T).astype(np.float16), D_HEAD)
    # w2r[oc, r, 128*ic + o] = W2T[128ic+r, 128oc+o]
    w2T = np.ascontiguousarray(W2.T).astype(np.float16)
    w2r = np.ascontiguousarray(
        w2T.reshape(NDM, 128, NDM, 128).transpose(2, 1, 0, 3).reshape(NDM, 128, D_MODEL)
    )

    in_maps = []
    for c in range(NCORES):
        xqT = np.ascontiguousarray(x[c::NCORES].T).astype(np.float16)  # [D, QPC]
        xqr_c = pack_chunks(xqT, QPC)
        mask = np.zeros((128, 16), dtype=np.float32)
        j = np.arange(128)[:, None]
        t = np.arange(16)[None, :]
        mask[j > 8 * t + c] = MASK_NEG
        in_maps.append(
            {
                "xqr": xqr_c,
                "xtp": xtp,
                "xv": xv16,
                "wqr": wqr,
                "wkr": wkr,
                "w2r": w2r,
                "maskb": mask,
            }
        )
    return in_maps


def assemble_output(results):
    res = np.stack([np.asarray(results[c]["outT"]).astype(np.float32) for c in range(NCORES)])
    # [c, d, m] -> out[8m+c, d]
    return np.ascontiguousarray(res.transpose(2, 0, 1).reshape(N_CTX, D_MODEL))


_CACHED = {}


def kernel(x, Wk, Wq, W2, _trace=False):
    if "nc" not in _CACHED:
        _CACHED["nc"] = build_program()
    nc = _CACHED["nc"]
    in_maps = prepare_inputs(x, Wk, Wq, W2)
    res = run_bass_kernel_spmd(nc, in_maps, core_ids=list(range(NCORES)), trace=_trace)
    out = assemble_output(res.results)
    if _trace:
        return out, res
    return out


# revision 11
# speedup vs baseline: 1.4492x; 1.1059x over previous
"""Causal attention kernel for TRN2, 8 NeuronCores (SPMD) — v3.

Problem:  x[4096,2048] f32; q = x@Wq.T, k = x@Wk.T (d_head=128),
          scores = q@k.T causal-masked, attn = softmax(scores),
          out = (attn @ x) @ W2.T.

Sharding: sequence-parallel over queries with stride-8 interleave:
  core c owns queries {8m+c : m=0..511}.  For key tile kt (128 keys),
  every core has exactly 512-16*kt valid queries -- a contiguous tail
  slice of its query columns -- so the SPMD program is identical on all
  cores (no dynamic control flow, no collectives) and causal work is
  perfectly balanced.

Precision: fp16 inputs for the q/k projections and the score matmul
  (fp32 PSUM accumulation), unnormalized softmax (exp without
  max-subtraction), attention weights in bf16 (fp32 exponent range,
  needed for exp(s) up to ~1e28), V and W2 matmuls in bf16/fp16,
  normalization by the softmax row-sum applied at eviction.

v3 scheduling (vs the 215us v1 baseline, trace-driven):
  * Dual-queue DMA: the xtp / xv / w2r streams alternate between the
    sync and scalar HWDGE queues (even tiles on sync, odd on scalar).
    One queue measured ~268 GB/s effective (issue-gap limited); two
    sequencers raise aggregate toward the ~358 GB/s HBM limit, which
    shortens the DMA-paced score window and starts the V phase
    earlier.  Per-queue emission order preserves the byte schedule
    (small loads -> xtp -> xv -> w2r) and xv stays ascending.
  * q-path small loads (wq, xqr) go on the scalar queue concurrently
    with wk+xtp0 on sync, so the kT projection starts ~5us earlier
    and the PE warms up immediately.
  * kT projection in 512-key groups (matmul N=512 instead of N=128):
    128 LDWEIGHTS+MM pairs instead of 512, saving ~13us of PE time,
    which matters once the score window is no longer DMA-bound.
  * Softmax reciprocal on the [128,512] broadcast grid (the [1,512]
    single-lane reciprocal was 3.3us of critical path).
  * V-matmul phase structure is v1's (PSUM-bank-optimal): fused oc0-3
    kt<16, g2 oc4-7 kt<16, bc oc8-15 full range, g4 oc0-7 kt>=16 with
    normalized merges; W2 interleaves via ic_order 8-15 first.
"""

from contextlib import ExitStack

import numpy as np
import ml_dtypes

import concourse.bass as bass
import concourse.bacc as bacc
import concourse.mybir as mybir
import concourse.tile as tile
from concourse.bass_utils import run_bass_kernel_spmd

N_CTX = 4096
D_MODEL = 2048
D_HEAD = 128
NCORES = 8
QPC = N_CTX // NCORES          # 512 queries per core
NKT = N_CTX // 128             # 32 key tiles
NDM = D_MODEL // 128           # 16 d_model chunks
SUBS = 4                       # key tiles per projection group
KG = SUBS * 128                # 512 keys per projection group
NKG = N_CTX // KG              # 8 projection groups
MASK_NEG = -1.0e30

F16 = mybir.dt.float16
BF16 = mybir.dt.bfloat16
F32 = mybir.dt.float32


def _widths():
    # valid query-column width per key tile (tail slice [512-w : 512] of qT)
    return [QPC - 16 * kt for kt in range(NKT)]


def build_program():
    nc = bacc.Bacc(trn_type="TRN2", target_bir_lowering=False, debug=False)

    # ---- DRAM parameters (identical shapes on all cores; data differs) ----
    # xqr[r, 512*ic + m] = x[8m+c, 128*ic + r]   (own-query columns, packed)
    xqr = nc.declare_dram_parameter("xqr", [128, NDM * QPC], F16, isOutput=False)
    # xtp[kg][r, KG*ic + n] = x[KG*kg + n, 128*ic + r]  (contiguous per-kg tiles)
    xtp = nc.declare_dram_parameter("xtp", [NKG, 128, NDM * KG], F16, isOutput=False)
    # xv = x (natural layout), bf16
    xv = nc.declare_dram_parameter("xv", [N_CTX, D_MODEL], BF16, isOutput=False)
    # wqr[r, 128*ic + h] = Wq[h, 128*ic + r]; same for wkr
    wqr = nc.declare_dram_parameter("wqr", [128, D_MODEL], F16, isOutput=False)
    wkr = nc.declare_dram_parameter("wkr", [128, D_MODEL], F16, isOutput=False)
    # w2r[oc][r, 128*ic + o] = W2[128*oc + o, 128*ic + r]
    w2r = nc.declare_dram_parameter("w2r", [NDM, 128, D_MODEL], F16, isOutput=False)
    maskb = nc.declare_dram_parameter("maskb", [128, 16], F32, isOutput=False)
    outT = nc.declare_dram_parameter("outT", [D_MODEL, QPC], F16, isOutput=True)

    W = _widths()

    def q_dma(i):
        # alternate big-stream tiles across the two HWDGE queues
        return nc.sync if i % 2 == 0 else nc.scalar

    with tile.TileContext(nc) as tc:
        with (
            tc.tile_pool(name="static", bufs=1) as st,
            tc.tile_pool(name="xvpool", bufs=NKT) as xvp,
            tc.tile_pool(name="atpool", bufs=1) as atp,
        ):
            qT_sb = st.tile([128, QPC], F16, tag="qT")
            ones_sb = st.tile([128, 1], BF16, tag="ones")
            mask_sb = st.tile([128, 16], F32, tag="mask")
            recip_sb = st.tile([128, QPC], F32, tag="recip")
            nc.vector.memset(ones_sb[:], 1.0)

            # ---- critical small loads: k-path on sync, q-path on scalar
            # (concurrent queues), then the big streams alternate.
            nc.sync.dma_start(out=mask_sb[:], in_=maskb[:])

            es1 = ExitStack()  # SBUF transients: p1 + xts (freed before p34)
            p1 = es1.enter_context(tc.tile_pool(name="p1", bufs=1))
            # head bytes mirrored across the two queues (~1.5MB each) so the
            # alternating xtp/xv tiles land in lockstep
            wk_sb = p1.tile([128, D_MODEL], F16, tag="wk")
            nc.sync.dma_start(out=wk_sb[:], in_=wkr[:])
            wq_sb = p1.tile([128, D_MODEL], F16, tag="wq")
            nc.scalar.dma_start(out=wq_sb[:], in_=wqr[:])
            xq_sb = p1.tile([128, NDM * QPC], F16, tag="xq")
            for qq in range(4):
                q_dma(qq).dma_start(
                    out=xq_sb[:, 4 * QPC * qq : 4 * QPC * (qq + 1)],
                    in_=xqr[:, 4 * QPC * qq : 4 * QPC * (qq + 1)],
                )

            # ---- xT stream: one contiguous 2MB DMA per 512-key group,
            # alternating queues; kg0 is split into 0.5MB quarters (= 4
            # d_model chunks each) so its kT projection starts early ----
            xts = es1.enter_context(tc.tile_pool(name="xts", bufs=2))
            ktp = es1.enter_context(tc.tile_pool(name="ktp", bufs=2))
            xts_t = []
            for kg in range(NKG):
                t = xts.tile([128, NDM * KG], F16, tag="xts", name=f"xts{kg}")
                if kg == 0:
                    for i in range(4):
                        nc.sync.dma_start(
                            out=t[:, 4 * KG * i : 4 * KG * (i + 1)],
                            in_=xtp[0][:, 4 * KG * i : 4 * KG * (i + 1)],
                        )
                else:
                    q_dma(kg).dma_start(out=t[:], in_=xtp[kg])
                xts_t.append(t)

            # ---- xv stream: after xtp per queue (emission order), ascending
            xv_t = []
            for kt in range(NKT):
                t = xvp.tile([128, D_MODEL], BF16, tag="xv", name=f"xv{kt}")
                q_dma(kt).dma_start(out=t[:], in_=xv[128 * kt : 128 * (kt + 1), :])
                xv_t.append(t)

            # ---- qT projection ----
            with tc.tile_pool(name="psq", bufs=1, space="PSUM") as psqp:
                psq = psqp.tile([128, QPC], F32, tag="psq")
                for ic in range(NDM):
                    nc.tensor.matmul(
                        psq[:],
                        wq_sb[:, 128 * ic : 128 * (ic + 1)],
                        xq_sb[:, QPC * ic : QPC * (ic + 1)],
                        start=(ic == 0), stop=(ic == NDM - 1),
                    )
                nc.vector.tensor_copy(qT_sb[:], psq[:])

            # ---- score pipeline: kT proj / scores / exp / denom ----
            # (V matmuls are emitted in a separate loop AFTER this one: their
            # xv inputs land later than the score inputs, and interleaving
            # them here head-of-line-blocks the PE stream on xv arrival)
            at_t = []
            with (
                tc.tile_pool(name="psk", bufs=2, space="PSUM") as pskp,
                tc.tile_pool(name="pss", bufs=2, space="PSUM") as pssp,
                tc.tile_pool(name="psd", bufs=1, space="PSUM") as psdp,
            ):
                psd = psdp.tile([1, QPC], F32, tag="psd")
                for kg in range(NKG):
                    psk = pskp.tile([128, KG], F32, tag="psk", name=f"psk{kg}")
                    for ic in range(NDM):
                        nc.tensor.matmul(
                            psk[:],
                            wk_sb[:, 128 * ic : 128 * (ic + 1)],
                            xts_t[kg][:, KG * ic : KG * (ic + 1)],
                            start=(ic == 0), stop=(ic == NDM - 1),
                        )
                    ktile = ktp.tile([128, KG], F16, tag="ktile", name=f"kt{kg}")
                    nc.vector.tensor_copy(ktile[:], psk[:])

                    for sub in range(SUBS):
                        kt = kg * SUBS + sub
                        w = W[kt]
                        ps = pssp.tile([128, 512], F32, tag="pss", name=f"pss{kt}")
                        nc.tensor.matmul(
                            ps[:, :w],
                            ktile[:, 128 * sub : 128 * (sub + 1)],
                            qT_sb[:, QPC - w : QPC],
                            start=True, stop=True,
                        )
                        nc.vector.tensor_add(ps[:, :16], ps[:, :16], mask_sb[:])
                        at = atp.tile([128, w], BF16, tag=f"at{kt}")
                        nc.scalar.activation(
                            at[:], ps[:, :w], mybir.ActivationFunctionType.Exp
                        )
                        at_t.append(at)
                        nc.tensor.matmul(
                            psd[0:1, QPC - w : QPC],
                            ones_sb[:],
                            at[:],
                            start=(kt == 0), stop=(kt == NKT - 1),
                        )

                # softmax denominators -> SBUF (reads psd before the pool
                # closes); broadcast + full-grid reciprocal follow
                nc.vector.tensor_copy(recip_sb[0:1, :], psd[0:1, :])

            es1.close()  # free p1 + xts SBUF for the aoT / W2 pools
            nc.gpsimd.partition_broadcast(recip_sb[:], recip_sb[0:1, :])
            nc.vector.reciprocal(recip_sb[:], recip_sb[:])

            with tc.tile_pool(name="p34", bufs=1) as p34:
                ao_t = {}

                # ---- oc 0-7, first key half (kt 0-15): 8 concurrent streams
                # on all 8 banks, paced by the xv arrivals ----
                HK = NKT // 2
                with (
                    tc.tile_pool(name="g2L", bufs=4, space="PSUM") as g2L,
                    tc.tile_pool(name="g2R", bufs=4, space="PSUM", side="right") as g2R,
                ):
                    g2 = {}
                    for oc in range(8):
                        pool = g2L if oc < 4 else g2R
                        g2[oc] = pool.tile(
                            [128, QPC], F32, tag="g2", name=f"g2_{oc}"
                        )
                    for kt in range(HK):
                        w = W[kt]
                        for oc in range(8):
                            nc.tensor.matmul(
                                g2[oc][:, QPC - w : QPC],
                                xv_t[kt][:, 128 * oc : 128 * (oc + 1)],
                                at_t[kt][:],
                                start=(kt == 0), stop=(kt == HK - 1),
                            )
                    # early normalized evictions for oc 0-7 (partial over the
                    # first key half; exact for queries m<256 by causality)
                    for oc in range(8):
                        t = p34.tile([128, QPC], F16, tag=f"ao{oc}")
                        nc.vector.tensor_mul(t[:], g2[oc][:], recip_sb[:])
                        ao_t[oc] = t

                # ---- oc 8-15, FULL key range: 8 concurrent streams on the
                # freed banks; kt 0-15 runs dense from resident xv while the
                # xv tail streams in ----
                with (
                    tc.tile_pool(name="bcL", bufs=4, space="PSUM") as bcL,
                    tc.tile_pool(name="bcR", bufs=4, space="PSUM", side="right") as bcR,
                ):
                    bc = {}
                    for i, oc in enumerate(range(8, NDM)):
                        pool = bcL if i < 4 else bcR
                        bc[oc] = pool.tile(
                            [128, QPC], F32, tag="bc", name=f"bc_{oc}"
                        )
                    for kt in range(NKT):
                        w = W[kt]
                        for oc in range(8, NDM):
                            nc.tensor.matmul(
                                bc[oc][:, QPC - w : QPC],
                                xv_t[kt][:, 128 * oc : 128 * (oc + 1)],
                                at_t[kt][:],
                                start=(kt == 0), stop=(kt == NKT - 1),
                            )
                    for oc in range(8, NDM):
                        t = p34.tile([128, QPC], F16, tag=f"ao{oc}")
                        nc.vector.tensor_mul(t[:], bc[oc][:], recip_sb[:])
                        ao_t[oc] = t

                # ---- oc 0-7, second key half (kt 16-31, queries [256:512]
                # only): dense from resident xv, merged into the early aos ----
                with (
                    tc.tile_pool(name="g4", bufs=4, space="PSUM") as g4p,
                    tc.tile_pool(name="tmr", bufs=4) as tmr,
                ):
                    for ocs in (range(0, 4), range(4, 8)):
                        g4 = {
                            oc: g4p.tile(
                                [128, QPC // 2], F32, tag="g4", name=f"g4_{oc}"
                            )
                            for oc in ocs
                        }
                        for kt in range(HK, NKT):
                            w = W[kt]
                            for oc in ocs:
                                nc.tensor.matmul(
                                    g4[oc][:, QPC // 2 - w : QPC // 2],
                                    xv_t[kt][:, 128 * oc : 128 * (oc + 1)],
                                    at_t[kt][:],
                                    start=(kt == HK), stop=(kt == NKT - 1),
                                )
                        for oc in ocs:
                            tm = tmr.tile([128, QPC // 2], F16, tag="tm")
                            nc.vector.tensor_mul(
                                tm[:], g4[oc][:], recip_sb[:, QPC // 2 :]
                            )
                            nc.vector.tensor_add(
                                ao_t[oc][:, QPC // 2 :],
                                ao_t[oc][:, QPC // 2 :],
                                tm[:],
                            )

                    # ---- W2: outT = W2T.T @ attn_outT.  ic order 8..15 first
                    # (those aos finish earliest), 0..7 after the merges ----
                    with (
                        tc.tile_pool(name="w2s", bufs=4) as w2s,
                        tc.tile_pool(name="outs", bufs=8) as outs,
                        tc.tile_pool(name="ps4", bufs=4, space="PSUM", side="right") as ps4,
                    ):
                        ic_order = list(range(8, NDM)) + list(range(0, 8))
                        for oc in range(NDM):
                            tw = w2s.tile([128, D_MODEL], F16, tag="w2")
                            q_dma(oc).dma_start(out=tw[:], in_=w2r[oc])
                            ps = ps4.tile([128, QPC], F32, tag="ps4")
                            for i, ic in enumerate(ic_order):
                                nc.tensor.matmul(
                                    ps[:],
                                    tw[:, 128 * ic : 128 * (ic + 1)],
                                    ao_t[ic][:],
                                    start=(i == 0), stop=(i == NDM - 1),
                                )
                            t = outs.tile([128, QPC], F16, tag="out")
                            nc.vector.tensor_copy(t[:], ps[:])
                            nc.scalar.dma_start(
                                out=outT[128 * oc : 128 * (oc + 1), :], in_=t[:]
                            )

    nc.compile()
    return nc


def prepare_inputs(x, Wk, Wq, W2):
    """Host-side sharding/layout prep. Returns in_maps for the 8 cores."""
    x = np.asarray(x, dtype=np.float32)
    Wk = np.asarray(Wk, dtype=np.float32)
    Wq = np.asarray(Wq, dtype=np.float32)
    W2 = np.asarray(W2, dtype=np.float32)

    xT16 = np.ascontiguousarray(x.T).astype(np.float16)          # [D, N]
    # xtp[kg, r, KG*ic + n] = xT[128*ic + r, KG*kg + n]
    xtp = np.ascontiguousarray(
        xT16.reshape(NDM, 128, NKG, KG).transpose(2, 1, 0, 3).reshape(NKG, 128, NDM * KG)
    )
    xv16 = x.astype(ml_dtypes.bfloat16)                          # [N, D]

    def pack_chunks(aT, width):
        # aT [D_MODEL, width] -> [128, NDM*width]: out[r, width*ic + c] = aT[128ic+r, c]
        return np.ascontiguousarray(
            aT.reshape(NDM, 128, width).transpose(1, 0, 2).reshape(128, NDM * width)
        )

    wqr = pack_chunks(np.ascontiguousarray(Wq.T).astype(np.float16), D_HEAD)
    wkr = pack_chunks(np.ascontiguousarray(Wk.T).astype(np.float16), D_HEAD)
    # w2r[oc, r, 128*ic + o] = W2T[128ic+r, 128oc+o]
    w2T = np.ascontiguousarray(W2.T).astype(np.float16)
    w2r = np.ascontiguousarray(
        w2T.reshape(NDM, 128, NDM, 128).transpose(2, 1, 0, 3).reshape(NDM, 128, D_MODEL)
    )

    in_maps = []
    for c in range(NCORES):
        xqT = np.ascontiguousarray(x[c::NCORES].T).astype(np.float16)  # [D, QPC]
        xqr_c = pack_chunks(xqT, QPC)
        mask = np.zeros((128, 16), dtype=np.float32)
        j = np.arange(128)[:, None]
        t = np.arange(16)[None, :]
        mask[j > 8 * t + c] = MASK_NEG
        in_maps.append(
            {
                "xqr": xqr_c,
                "xtp": xtp,
                "xv": xv16,
                "wqr": wqr,
                "wkr": wkr,
                "w2r": w2r,
                "maskb": mask,
            }
        )
    return in_maps


def assemble_output(results):
    res = np.stack([np.asarray(results[c]["outT"]).astype(np.float32) for c in range(NCORES)])
    # [c, d, m] -> out[8m+c, d]
    return np.ascontiguousarray(res.transpose(2, 0, 1).reshape(N_CTX, D_MODEL))


_CACHED = {}


def kernel(x, Wk, Wq, W2, _trace=False):
    if "nc" not in _CACHED:
        _CACHED["nc"] = build_program()
    nc = _CACHED["nc"]
    in_maps = prepare_inputs(x, Wk, Wq, W2)
    res = run_bass_kernel_spmd(nc, in_maps, core_ids=list(range(NCORES)), trace=_trace)
    out = assemble_output(res.results)
    if _trace:
        return out, res
    return out
